# revision 1
# baseline (speedup 1.0000x reference)
"""Trainium2 Bass kernel for nn_ConsitencyLoss (8 NeuronCores, data parallel).

reference semantics:
    row_mask  = seg_weight != 0                                  # [B]
    chan_keep = arange(C)[None,:] != seg_weight[:,None]          # [B, C]
    mask      = row_mask[:,None] & chan_keep                     # [B, C]
    out = sum(sigmoid(inputs) * mask[:,:,None,None])
          / (row_mask.sum() * H*W*C + 1)

Strategy: mask[b,c] is 0/1 and computable on the host from seg_weight, so only
the *kept* (b,c) planes (HxW each) are shipped to the device — for the seed-0
draw that is 82 of 192 planes, a 2.3x HBM-traffic cut. Kept planes are packed
and balanced round-robin across the 8 cores (P = ceil(K/8) planes per core,
zero-padded); every core runs the same NEFF:

    for each plane: DMA [128, H*W/128] HBM->SBUF (sync ring, 4-deep pool),
                    ScalarE ACTIVATE(Sigmoid, accum_out) -> per-partition sums
    one final DMA of the [128, P] accumulator to HBM.

The single ACTIVATE per plane computes sigmoid AND its free-dim sum in one
pass, so ScalarE (1.79us/plane) stays under the DMA (2.6us/plane) and the
kernel is DMA-bound end to end. Measured on HW: 345 GB/s/core steady-state =
96% of the 358 GB/s per-core HBM roofline.

The host finishes with the tiny [8P] per-plane reduction in float64, applies
the pad-plane mask, and divides by the count-derived denominator.
"""
import numpy as np

NCORES = 8

# (P, FREE) -> dict with the cached jitted runner and metadata
_RUNNERS: dict = {}


def _build_nc(P: int, FREE: int):
    import concourse.bacc as bacc
    import concourse.mybir as mybir
    import concourse.tile as tile

    nc = bacc.Bacc(
        "TRN2",
        target_bir_lowering=False,
        debug=False,
        enable_asserts=False,
        enable_partition_id=False,
        num_devices=NCORES,
    )
    x = nc.dram_tensor("x", [P, 128, FREE], mybir.dt.float32, kind="ExternalInput").ap()
    o = nc.dram_tensor("o", [128, P], mybir.dt.float32, kind="ExternalOutput").ap()
    with tile.TileContext(nc) as tc:
        with tc.tile_pool(name="sbuf", bufs=4) as pool, tc.tile_pool(
            name="accp", bufs=1
        ) as accp:
            acc = accp.tile([128, P], mybir.dt.float32)
            for i in range(P):
                t = pool.tile([128, FREE], mybir.dt.float32)
                nc.sync.dma_start(t, x[i])
                nc.scalar.activation(
                    t,
                    t,
                    mybir.ActivationFunctionType.Sigmoid,
                    accum_out=acc[:, i : i + 1],
                )
            nc.sync.dma_start(o, acc)
    nc.compile()
    return nc


def _make_cached_runner(P: int, FREE: int):
    """Jitted shard_map runner mirroring concourse.bass2jax.run_bass_via_pjrt's
    multi-core path (the axon redirect target of bass_utils.run_bass_kernel_spmd)
    but reusable across calls, so repeated kernel() invocations don't re-jit."""
    import jax
    from jax.experimental.shard_map import shard_map
    from jax.sharding import Mesh, PartitionSpec

    import concourse.mybir as mybir
    from concourse.bass2jax import _bass_exec_p, install_neuronx_cc_hook

    nc = _build_nc(P, FREE)
    install_neuronx_cc_hook()
    assert nc.partition_id_tensor is None and nc.dbg_addr is None

    in_names, out_names, out_avals = [], [], []
    for alloc in nc.m.functions[0].allocations:
        if not isinstance(alloc, mybir.MemoryLocationSet):
            continue
        name = alloc.memorylocations[0].name
        if alloc.kind == "ExternalInput":
            in_names.append(name)
        elif alloc.kind == "ExternalOutput":
            out_names.append(name)
            out_avals.append(
                jax.core.ShapedArray(
                    tuple(alloc.tensor_shape), mybir.dt.np(alloc.dtype)
                )
            )
    n_params = len(in_names)
    n_outs = len(out_names)
    all_names = tuple(in_names + out_names)

    def _body(*args):
        outs = _bass_exec_p.bind(
            *args,
            out_avals=tuple(out_avals),
            in_names=all_names,
            out_names=tuple(out_names),
            lowering_input_output_aliases=(),
            sim_require_finite=True,
            sim_require_nnan=True,
            nc=nc,
        )
        return tuple(outs)

    mesh = Mesh(np.asarray(jax.devices()[:NCORES]), ("core",))
    fn = jax.jit(
        shard_map(
            _body,
            mesh=mesh,
            in_specs=(PartitionSpec("core"),) * (n_params + n_outs),
            out_specs=(PartitionSpec("core"),) * n_outs,
            check_rep=False,
        ),
        donate_argnums=tuple(range(n_params, n_params + n_outs)),
        keep_unused=True,
    )

    def run(packed: np.ndarray) -> np.ndarray:
        """packed: [NCORES*P, 128, FREE] f32 -> [NCORES*128, P] f32."""
        zeros = [
            np.zeros((NCORES * av.shape[0], *av.shape[1:]), av.dtype)
            for av in out_avals
        ]
        outs = fn(packed, *zeros)
        return np.asarray(outs[0])

    return run


def _run_packed(P: int, FREE: int, packed: np.ndarray) -> np.ndarray:
    key = (P, FREE)
    runner = _RUNNERS.get(key)
    if runner is None:
        try:
            runner = _make_cached_runner(P, FREE)
        except Exception:
            runner = None
        _RUNNERS[key] = runner
    if runner is not None:
        return runner(packed)
    # Fallback: the stock SPMD entry point (fresh jit per call).
    from concourse.bass_utils import run_bass_kernel_spmd

    nc = _build_nc(P, FREE)
    in_maps = [{"x": packed[j * P : (j + 1) * P]} for j in range(NCORES)]
    res = run_bass_kernel_spmd(nc, in_maps, core_ids=list(range(NCORES)))
    return np.concatenate([res.results[j]["o"] for j in range(NCORES)], axis=0)


def kernel(inputs: np.ndarray, seg_weight: np.ndarray) -> np.ndarray:
    inputs = np.asarray(inputs)
    if inputs.dtype != np.float32:
        inputs = inputs.astype(np.float32)
    sw = np.asarray(seg_weight).astype(np.int64).ravel()

    B, C, H, W = inputs.shape
    row = sw != 0
    keep = row[:, None] & (np.arange(C)[None, :] != sw[:, None])  # [B, C]
    denom = float(row.sum()) * float(H * W * C) + 1.0

    K = int(keep.sum())
    if K == 0:
        return np.asarray(0.0, dtype=np.float32)

    assert (H * W) % 128 == 0, (H, W)
    FREE = (H * W) // 128
    P = -(-K // NCORES)  # ceil

    packed = np.zeros((NCORES * P, 128, FREE), np.float32)
    packed[:K] = inputs[keep].reshape(K, 128, FREE)

    out = _run_packed(P, FREE, packed)  # [NCORES*128, P]
    # per-plane totals, cores-major order matching `packed`
    plane_sums = (
        out.reshape(NCORES, 128, P).sum(axis=1, dtype=np.float64).reshape(NCORES * P)
    )
    total = plane_sums[:K].sum()
    return np.asarray(np.float32(total / denom))


# revision 2
# speedup vs baseline: 1.0245x; 1.0245x over previous
"""Trainium2 Bass kernel for nn_ConsitencyLoss (8 NeuronCores, data parallel).

reference semantics:
    row_mask  = seg_weight != 0                                  # [B]
    chan_keep = arange(C)[None,:] != seg_weight[:,None]          # [B, C]
    mask      = row_mask[:,None] & chan_keep                     # [B, C]
    out = sum(sigmoid(inputs) * mask[:,:,None,None])
          / (row_mask.sum() * H*W*C + 1)

Strategy: mask[b,c] is 0/1 and computable on the host from seg_weight, so only
the *kept* (b,c) planes are shipped to the device — for the seed-0 draw that
is 82 of 192 planes, a 2.3x HBM-traffic cut. All kept elements are packed into
one flat stream, zero-padded, and split into 8 exactly equal per-core shards
(perfect load balance, no per-plane granularity needed since every shipped
element has mask 1; the host subtracts the pads' exact sigmoid(0)=0.5
contribution afterwards). Every core runs the same NEFF over its shard laid
out as Q contiguous blocks of [128, T] (~1 MiB each):

    for each block: DMA HBM->SBUF (sync-engine HWDGE ring, 4-deep tile pool),
                    ScalarE ACTIVATE(Sigmoid, accum_out) -> per-partition sums
    one final DMA of the [128, Q] accumulator to HBM.

The single ACTIVATE per block computes sigmoid AND its free-dim sum in one
pass, so ScalarE (~17us) stays under the DMA stream (~27us) and the kernel is
DMA-bound end to end. Measured on HW: ~340 GB/s/core steady-state = ~95% of
the 358 GB/s per-core HBM roofline.

The host finishes with the tiny [8*128, Q] reduction in float64 and divides
by the count-derived denominator.
"""
import numpy as np

NCORES = 8
TARGET_COLS = 2048  # aim for ~1 MiB per DMA ([128, 2048] f32)

# (Q, T) -> cached jitted runner (or None if the cached path failed)
_RUNNERS: dict = {}


def _build_nc(Q: int, T: int):
    import concourse.bacc as bacc
    import concourse.mybir as mybir
    import concourse.tile as tile

    nc = bacc.Bacc(
        "TRN2",
        target_bir_lowering=False,
        debug=False,
        enable_asserts=False,
        enable_partition_id=False,
        num_devices=NCORES,
    )
    x = nc.dram_tensor("x", [Q, 128, T], mybir.dt.float32, kind="ExternalInput").ap()
    o = nc.dram_tensor("o", [128, Q], mybir.dt.float32, kind="ExternalOutput").ap()
    with tile.TileContext(nc) as tc:
        with tc.tile_pool(name="sbuf", bufs=4) as pool, tc.tile_pool(
            name="accp", bufs=1
        ) as accp:
            acc = accp.tile([128, Q], mybir.dt.float32)
            for j in range(Q):
                t = pool.tile([128, T], mybir.dt.float32)
                nc.sync.dma_start(t, x[j])
                nc.scalar.activation(
                    t,
                    t,
                    mybir.ActivationFunctionType.Sigmoid,
                    accum_out=acc[:, j : j + 1],
                )
            nc.sync.dma_start(o, acc)
    nc.compile()
    return nc


def _make_cached_runner(Q: int, T: int):
    """Jitted shard_map runner mirroring concourse.bass2jax.run_bass_via_pjrt's
    multi-core path (the axon redirect target of bass_utils.run_bass_kernel_spmd)
    but reusable across calls, so repeated kernel() invocations don't re-jit."""
    import jax
    from jax.experimental.shard_map import shard_map
    from jax.sharding import Mesh, PartitionSpec

    import concourse.mybir as mybir
    from concourse.bass2jax import _bass_exec_p, install_neuronx_cc_hook

    nc = _build_nc(Q, T)
    install_neuronx_cc_hook()
    assert nc.partition_id_tensor is None and nc.dbg_addr is None

    in_names, out_names, out_avals = [], [], []
    for alloc in nc.m.functions[0].allocations:
        if not isinstance(alloc, mybir.MemoryLocationSet):
            continue
        name = alloc.memorylocations[0].name
        if alloc.kind == "ExternalInput":
            in_names.append(name)
        elif alloc.kind == "ExternalOutput":
            out_names.append(name)
            out_avals.append(
                jax.core.ShapedArray(
                    tuple(alloc.tensor_shape), mybir.dt.np(alloc.dtype)
                )
            )
    n_params = len(in_names)
    n_outs = len(out_names)
    all_names = tuple(in_names + out_names)

    def _body(*args):
        outs = _bass_exec_p.bind(
            *args,
            out_avals=tuple(out_avals),
            in_names=all_names,
            out_names=tuple(out_names),
            lowering_input_output_aliases=(),
            sim_require_finite=True,
            sim_require_nnan=True,
            nc=nc,
        )
        return tuple(outs)

    mesh = Mesh(np.asarray(jax.devices()[:NCORES]), ("core",))
    fn = jax.jit(
        shard_map(
            _body,
            mesh=mesh,
            in_specs=(PartitionSpec("core"),) * (n_params + n_outs),
            out_specs=(PartitionSpec("core"),) * n_outs,
            check_rep=False,
        ),
        donate_argnums=tuple(range(n_params, n_params + n_outs)),
        keep_unused=True,
    )

    def run(packed: np.ndarray) -> np.ndarray:
        """packed: [NCORES*Q, 128, T] f32 -> [NCORES*128, Q] f32."""
        zeros = [
            np.zeros((NCORES * av.shape[0], *av.shape[1:]), av.dtype)
            for av in out_avals
        ]
        outs = fn(packed, *zeros)
        return np.asarray(outs[0])

    return run


def _run_packed(Q: int, T: int, packed: np.ndarray) -> np.ndarray:
    key = (Q, T)
    runner = _RUNNERS.get(key)
    if runner is None and key not in _RUNNERS:
        try:
            runner = _make_cached_runner(Q, T)
        except Exception:
            runner = None
        _RUNNERS[key] = runner
    if runner is not None:
        return runner(packed)
    # Fallback: the stock SPMD entry point (fresh jit per call).
    from concourse.bass_utils import run_bass_kernel_spmd

    nc = _build_nc(Q, T)
    in_maps = [{"x": packed[j * Q : (j + 1) * Q]} for j in range(NCORES)]
    res = run_bass_kernel_spmd(nc, in_maps, core_ids=list(range(NCORES)))
    return np.concatenate([res.results[j]["o"] for j in range(NCORES)], axis=0)


def kernel(inputs: np.ndarray, seg_weight: np.ndarray) -> np.ndarray:
    inputs = np.asarray(inputs)
    if inputs.dtype != np.float32:
        inputs = inputs.astype(np.float32)
    sw = np.asarray(seg_weight).astype(np.int64).ravel()

    B, C, H, W = inputs.shape
    row = sw != 0
    keep = row[:, None] & (np.arange(C)[None, :] != sw[:, None])  # [B, C]
    denom = float(row.sum()) * float(H * W * C) + 1.0

    K = int(keep.sum())
    if K == 0:
        return np.asarray(0.0, dtype=np.float32)

    E = K * H * W  # real element count
    cols = -(-E // (NCORES * 128))  # per-core columns, ceil
    Q = max(1, -(-cols // TARGET_COLS))
    T = -(-cols // Q)
    cap = NCORES * Q * 128 * T
    n_pad = cap - E

    packed = np.zeros(cap, np.float32)  # pads are 0 -> sigmoid contributes 0.5
    packed[:E] = inputs[keep].ravel()

    out = _run_packed(Q, T, packed.reshape(NCORES * Q, 128, T))  # [8*128, Q]
    total = out.sum(dtype=np.float64) - 0.5 * n_pad
    return np.asarray(np.float32(total / denom))


# revision 3
# speedup vs baseline: 1.0259x; 1.0014x over previous
"""Trainium2 Bass kernel for nn_ConsitencyLoss (8 NeuronCores, data parallel).

reference semantics:
    row_mask  = seg_weight != 0                                  # [B]
    chan_keep = arange(C)[None,:] != seg_weight[:,None]          # [B, C]
    mask      = row_mask[:,None] & chan_keep                     # [B, C]
    out = sum(sigmoid(inputs) * mask[:,:,None,None])
          / (row_mask.sum() * H*W*C + 1)

Strategy: mask[b,c] is 0/1 and computable on the host from seg_weight, so only
the *kept* (b,c) planes are shipped to the device — for the seed-0 draw that
is 82 of 192 planes, a 2.3x HBM-traffic cut. All kept elements are packed into
one flat stream, zero-padded, and split into 8 exactly equal per-core shards
(perfect load balance, no per-plane granularity needed since every shipped
element has mask 1; the host subtracts the pads' exact sigmoid(0)=0.5
contribution afterwards). Every core runs the same NEFF over its shard laid
out as Q contiguous blocks of [128, T] (~1 MiB each):

    for each block: DMA HBM->SBUF (sync-engine HWDGE ring, 4-deep tile pool),
                    ScalarE ACTIVATE(Sigmoid, accum_out) -> per-partition sums
    one final DMA of the [128, Q] accumulator to HBM.

The single ACTIVATE per block computes sigmoid AND its free-dim sum in one
pass, so ScalarE (~17us) stays under the DMA stream (~27us) and the kernel is
DMA-bound end to end. Measured on HW: ~340 GB/s/core steady-state = ~95% of
the 358 GB/s per-core HBM roofline.

The host finishes with the tiny [8*128, Q] reduction in float64 and divides
by the count-derived denominator.
"""
import numpy as np

NCORES = 8
TARGET_COLS = 2048  # aim for ~1 MiB per DMA ([128, 2048] f32)

# (Q, T) -> cached jitted runner (or None if the cached path failed)
_RUNNERS: dict = {}


def _build_nc(Q: int, T: int):
    import concourse.bacc as bacc
    import concourse.mybir as mybir
    import concourse.tile as tile

    nc = bacc.Bacc(
        "TRN2",
        target_bir_lowering=False,
        debug=False,
        enable_asserts=False,
        enable_partition_id=False,
        num_devices=NCORES,
    )
    x = nc.dram_tensor("x", [Q, 128, T], mybir.dt.float32, kind="ExternalInput").ap()
    o = nc.dram_tensor("o", [128, Q], mybir.dt.float32, kind="ExternalOutput").ap()
    # Deep prefetch (all DMAs queued up front, every tile resident) streams
    # ~3% faster than a rolling 4-deep pool; use it whenever SBUF fits.
    deep = Q * T * 128 * 4 <= 20 * 2**20
    with tile.TileContext(nc) as tc:
        with tc.tile_pool(name="sbuf", bufs=Q if deep else 4) as pool, tc.tile_pool(
            name="accp", bufs=1
        ) as accp:
            acc = accp.tile([128, Q], mybir.dt.float32)
            if deep:
                tiles = []
                for j in range(Q):
                    t = pool.tile([128, T], mybir.dt.float32)
                    nc.sync.dma_start(t, x[j])
                    tiles.append(t)
                for j, t in enumerate(tiles):
                    nc.scalar.activation(
                        t,
                        t,
                        mybir.ActivationFunctionType.Sigmoid,
                        accum_out=acc[:, j : j + 1],
                    )
            else:
                for j in range(Q):
                    t = pool.tile([128, T], mybir.dt.float32)
                    nc.sync.dma_start(t, x[j])
                    nc.scalar.activation(
                        t,
                        t,
                        mybir.ActivationFunctionType.Sigmoid,
                        accum_out=acc[:, j : j + 1],
                    )
            nc.sync.dma_start(o, acc)
    nc.compile()
    return nc


def _make_cached_runner(Q: int, T: int):
    """Jitted shard_map runner mirroring concourse.bass2jax.run_bass_via_pjrt's
    multi-core path (the axon redirect target of bass_utils.run_bass_kernel_spmd)
    but reusable across calls, so repeated kernel() invocations don't re-jit."""
    import jax
    from jax.experimental.shard_map import shard_map
    from jax.sharding import Mesh, PartitionSpec

    import concourse.mybir as mybir
    from concourse.bass2jax import _bass_exec_p, install_neuronx_cc_hook

    nc = _build_nc(Q, T)
    install_neuronx_cc_hook()
    assert nc.partition_id_tensor is None and nc.dbg_addr is None

    in_names, out_names, out_avals = [], [], []
    for alloc in nc.m.functions[0].allocations:
        if not isinstance(alloc, mybir.MemoryLocationSet):
            continue
        name = alloc.memorylocations[0].name
        if alloc.kind == "ExternalInput":
            in_names.append(name)
        elif alloc.kind == "ExternalOutput":
            out_names.append(name)
            out_avals.append(
                jax.core.ShapedArray(
                    tuple(alloc.tensor_shape), mybir.dt.np(alloc.dtype)
                )
            )
    n_params = len(in_names)
    n_outs = len(out_names)
    all_names = tuple(in_names + out_names)

    def _body(*args):
        outs = _bass_exec_p.bind(
            *args,
            out_avals=tuple(out_avals),
            in_names=all_names,
            out_names=tuple(out_names),
            lowering_input_output_aliases=(),
            sim_require_finite=True,
            sim_require_nnan=True,
            nc=nc,
        )
        return tuple(outs)

    mesh = Mesh(np.asarray(jax.devices()[:NCORES]), ("core",))
    fn = jax.jit(
        shard_map(
            _body,
            mesh=mesh,
            in_specs=(PartitionSpec("core"),) * (n_params + n_outs),
            out_specs=(PartitionSpec("core"),) * n_outs,
            check_rep=False,
        ),
        donate_argnums=tuple(range(n_params, n_params + n_outs)),
        keep_unused=True,
    )

    def run(packed: np.ndarray) -> np.ndarray:
        """packed: [NCORES*Q, 128, T] f32 -> [NCORES*128, Q] f32."""
        zeros = [
            np.zeros((NCORES * av.shape[0], *av.shape[1:]), av.dtype)
            for av in out_avals
        ]
        outs = fn(packed, *zeros)
        return np.asarray(outs[0])

    return run


def _run_packed(Q: int, T: int, packed: np.ndarray) -> np.ndarray:
    key = (Q, T)
    runner = _RUNNERS.get(key)
    if runner is None and key not in _RUNNERS:
        try:
            runner = _make_cached_runner(Q, T)
        except Exception:
            runner = None
        _RUNNERS[key] = runner
    if runner is not None:
        return runner(packed)
    # Fallback: the stock SPMD entry point (fresh jit per call).
    from concourse.bass_utils import run_bass_kernel_spmd

    nc = _build_nc(Q, T)
    in_maps = [{"x": packed[j * Q : (j + 1) * Q]} for j in range(NCORES)]
    res = run_bass_kernel_spmd(nc, in_maps, core_ids=list(range(NCORES)))
    return np.concatenate([res.results[j]["o"] for j in range(NCORES)], axis=0)


def kernel(inputs: np.ndarray, seg_weight: np.ndarray) -> np.ndarray:
    inputs = np.asarray(inputs)
    if inputs.dtype != np.float32:
        inputs = inputs.astype(np.float32)
    sw = np.asarray(seg_weight).astype(np.int64).ravel()

    B, C, H, W = inputs.shape
    row = sw != 0
    keep = row[:, None] & (np.arange(C)[None, :] != sw[:, None])  # [B, C]
    denom = float(row.sum()) * float(H * W * C) + 1.0

    K = int(keep.sum())
    if K == 0:
        return np.asarray(0.0, dtype=np.float32)

    E = K * H * W  # real element count
    cols = -(-E // (NCORES * 128))  # per-core columns, ceil
    Q = max(1, -(-cols // TARGET_COLS))
    T = -(-cols // Q)
    cap = NCORES * Q * 128 * T
    n_pad = cap - E

    packed = np.zeros(cap, np.float32)  # pads are 0 -> sigmoid contributes 0.5
    packed[:E] = inputs[keep].ravel()

    out = _run_packed(Q, T, packed.reshape(NCORES * Q, 128, T))  # [8*128, Q]
    total = out.sum(dtype=np.float64) - 0.5 * n_pad
    return np.asarray(np.float32(total / denom))


# revision 5
# speedup vs baseline: 1.0709x; 1.0438x over previous
"""Trainium2 Bass kernel for nn_ConsitencyLoss (8 NeuronCores, data parallel).

reference semantics:
    row_mask  = seg_weight != 0                                  # [B]
    chan_keep = arange(C)[None,:] != seg_weight[:,None]          # [B, C]
    mask      = row_mask[:,None] & chan_keep                     # [B, C]
    out = sum(sigmoid(inputs) * mask[:,:,None,None])
          / (row_mask.sum() * H*W*C + 1)

Strategy: mask[b,c] is 0/1 and computable on the host from seg_weight, so only
the *kept* (b,c) planes are shipped to the device — for the seed-0 draw that
is 82 of 192 planes, a 2.3x HBM-traffic cut. All kept elements are packed into
one flat stream, zero-padded, and split into 8 exactly equal per-core shards
(perfect load balance; no per-plane granularity is needed since every shipped
element has mask 1, and the host subtracts the pads' exact sigmoid(0)=0.5
contribution afterwards). Every core runs the same NEFF over its shard laid
out as Qb contiguous blocks of [128, TB] (~1 MiB) plus one smaller tail block
[128, Ts], Ts ~ 0.7*TB:

    all DMAs queued up front on the sync-engine HWDGE ring (deep prefetch,
    every tile resident — measured ~3% faster than a rolling pool), then one
    ScalarE ACTIVATE(Sigmoid, accum_out) per block -> per-partition sums,
    one final DMA of the [128, Q] accumulator to HBM.

The single ACTIVATE per block computes sigmoid AND its free-dim sum in one
pass, so ScalarE (~17us) stays under the DMA stream (~27us) and the kernel is
DMA-bound end to end. The smaller tail block shortens the post-stream drain
(last DMA -> sem -> last ACT), worth ~0.4us on HW. Timeline (cost model,
validated on HW): ~2us entry, ~26us DMA stream at roofline, ~2.4us ACT drain,
~2.9us exit barrier. Measured HW streaming: ~345 GB/s/core = 96% of the
358 GB/s per-core HBM limit.

The host finishes with the tiny [8*128, Q] reduction in float64 and divides
by the count-derived denominator.
"""
import numpy as np

NCORES = 8
TARGET_COLS = 2048   # aim for ~1 MiB per big-block DMA ([128, 2048] f32)
TAIL_FRAC = 0.707    # tail block ~0.7*TB minimizes the post-stream ACT drain
DEEP_SBUF_LIMIT = 20 * 2**20  # deep prefetch only if all tiles fit in SBUF

# (Qb, TB, Ts) -> cached jitted runner (or None if the cached path failed)
_RUNNERS: dict = {}


def _plan(cols: int):
    """Split per-core `cols` into Qb big blocks of TB + one tail of Ts."""
    if cols * 128 * 4 > DEEP_SBUF_LIMIT or cols <= 4096:
        # rolling-pool or small problem: uniform blocks, no tail
        Qb = max(1, -(-cols // TARGET_COLS))
        TB = -(-cols // Qb)
        return Qb, TB, 0
    Qb = max(1, round(cols / TARGET_COLS - TAIL_FRAC))
    TB = int(-(-cols * 1000 // int((Qb + TAIL_FRAC) * 1000)))
    TB = min(TB, cols // Qb)  # keep Qb*TB <= cols so Ts >= 0
    Ts = cols - Qb * TB
    if Ts == 0:
        return Qb, TB, 0
    return Qb, TB, Ts


def _build_nc(Qb: int, TB: int, Ts: int):
    import concourse.bacc as bacc
    import concourse.mybir as mybir
    import concourse.tile as tile

    Q = Qb + (1 if Ts else 0)
    nc = bacc.Bacc(
        "TRN2",
        target_bir_lowering=False,
        debug=False,
        enable_asserts=False,
        enable_partition_id=False,
        num_devices=NCORES,
    )
    xb = nc.dram_tensor("xb", [Qb, 128, TB], mybir.dt.float32, kind="ExternalInput").ap()
    xt = (
        nc.dram_tensor("xt", [128, Ts], mybir.dt.float32, kind="ExternalInput").ap()
        if Ts
        else None
    )
    o = nc.dram_tensor("o", [128, Q], mybir.dt.float32, kind="ExternalOutput").ap()
    deep = (Qb * TB + Ts) * 128 * 4 <= DEEP_SBUF_LIMIT
    with tile.TileContext(nc) as tc:
        with tc.tile_pool(name="sbuf", bufs=1 if deep else 4) as pool, tc.tile_pool(
            name="accp", bufs=1
        ) as accp:
            acc = accp.tile([128, Q], mybir.dt.float32)
            if deep:
                tiles = []
                for j in range(Qb):
                    t = pool.tile([128, TB], mybir.dt.float32, tag=f"b{j}")
                    nc.sync.dma_start(t, xb[j])
                    tiles.append(t)
                if Ts:
                    t = pool.tile([128, Ts], mybir.dt.float32, tag="tail")
                    nc.sync.dma_start(t, xt)
                    tiles.append(t)
                for j, t in enumerate(tiles):
                    nc.scalar.activation(
                        t,
                        t,
                        mybir.ActivationFunctionType.Sigmoid,
                        accum_out=acc[:, j : j + 1],
                    )
            else:
                for j in range(Qb):
                    t = pool.tile([128, TB], mybir.dt.float32, tag="roll")
                    nc.sync.dma_start(t, xb[j])
                    nc.scalar.activation(
                        t,
                        t,
                        mybir.ActivationFunctionType.Sigmoid,
                        accum_out=acc[:, j : j + 1],
                    )
                if Ts:
                    t = pool.tile([128, Ts], mybir.dt.float32, tag="tail")
                    nc.sync.dma_start(t, xt)
                    nc.scalar.activation(
                        t,
                        t,
                        mybir.ActivationFunctionType.Sigmoid,
                        accum_out=acc[:, Qb : Qb + 1],
                    )
            nc.sync.dma_start(o, acc)
    nc.compile()
    return nc


def _make_cached_runner(Qb: int, TB: int, Ts: int):
    """Jitted shard_map runner mirroring concourse.bass2jax.run_bass_via_pjrt's
    multi-core path (the axon redirect target of bass_utils.run_bass_kernel_spmd)
    but reusable across calls, so repeated kernel() invocations don't re-jit."""
    import jax
    from jax.experimental.shard_map import shard_map
    from jax.sharding import Mesh, PartitionSpec

    import concourse.mybir as mybir
    from concourse.bass2jax import _bass_exec_p, install_neuronx_cc_hook

    nc = _build_nc(Qb, TB, Ts)
    install_neuronx_cc_hook()
    assert nc.partition_id_tensor is None and nc.dbg_addr is None

    in_names, out_names, out_avals = [], [], []
    for alloc in nc.m.functions[0].allocations:
        if not isinstance(alloc, mybir.MemoryLocationSet):
            continue
        name = alloc.memorylocations[0].name
        if alloc.kind == "ExternalInput":
            in_names.append(name)
        elif alloc.kind == "ExternalOutput":
            out_names.append(name)
            out_avals.append(
                jax.core.ShapedArray(
                    tuple(alloc.tensor_shape), mybir.dt.np(alloc.dtype)
                )
            )
    n_params = len(in_names)
    n_outs = len(out_names)
    all_names = tuple(in_names + out_names)

    def _body(*args):
        outs = _bass_exec_p.bind(
            *args,
            out_avals=tuple(out_avals),
            in_names=all_names,
            out_names=tuple(out_names),
            lowering_input_output_aliases=(),
            sim_require_finite=True,
            sim_require_nnan=True,
            nc=nc,
        )
        return tuple(outs)

    mesh = Mesh(np.asarray(jax.devices()[:NCORES]), ("core",))
    fn = jax.jit(
        shard_map(
            _body,
            mesh=mesh,
            in_specs=(PartitionSpec("core"),) * (n_params + n_outs),
            out_specs=(PartitionSpec("core"),) * n_outs,
            check_rep=False,
        ),
        donate_argnums=tuple(range(n_params, n_params + n_outs)),
        keep_unused=True,
    )
    order = list(in_names)

    def run(arrs: dict) -> np.ndarray:
        """arrs: {"xb": [8*Qb,128,TB], "xt": [8*128,Ts]?} -> [8*128, Q]."""
        zeros = [
            np.zeros((NCORES * av.shape[0], *av.shape[1:]), av.dtype)
            for av in out_avals
        ]
        outs = fn(*[arrs[n] for n in order], *zeros)
        return np.asarray(outs[0])

    return run


def _run_packed(Qb: int, TB: int, Ts: int, arrs: dict) -> np.ndarray:
    key = (Qb, TB, Ts)
    if key not in _RUNNERS:
        try:
            _RUNNERS[key] = _make_cached_runner(Qb, TB, Ts)
        except Exception:
            _RUNNERS[key] = None
    runner = _RUNNERS[key]
    if runner is not None:
        return runner(arrs)
    # Fallback: the stock SPMD entry point (fresh jit per call).
    from concourse.bass_utils import run_bass_kernel_spmd

    nc = _build_nc(Qb, TB, Ts)
    in_maps = []
    for c in range(NCORES):
        m = {"xb": arrs["xb"][c * Qb : (c + 1) * Qb]}
        if Ts:
            m["xt"] = arrs["xt"][c * 128 : (c + 1) * 128]
        in_maps.append(m)
    res = run_bass_kernel_spmd(nc, in_maps, core_ids=list(range(NCORES)))
    return np.concatenate([res.results[j]["o"] for j in range(NCORES)], axis=0)


def kernel(inputs: np.ndarray, seg_weight: np.ndarray) -> np.ndarray:
    inputs = np.asarray(inputs)
    if inputs.dtype != np.float32:
        inputs = inputs.astype(np.float32)
    sw = np.asarray(seg_weight).astype(np.int64).ravel()

    B, C, H, W = inputs.shape
    row = sw != 0
    keep = row[:, None] & (np.arange(C)[None, :] != sw[:, None])  # [B, C]
    denom = float(row.sum()) * float(H * W * C) + 1.0

    K = int(keep.sum())
    if K == 0:
        return np.asarray(0.0, dtype=np.float32)

    E = K * H * W  # real element count
    cols = -(-E // (NCORES * 128))  # per-core columns, ceil
    Qb, TB, Ts = _plan(cols)
    per_core = (Qb * TB + Ts) * 128
    cap = NCORES * per_core
    n_pad = cap - E

    packed = np.zeros(cap, np.float32)  # pads are 0 -> sigmoid contributes 0.5
    packed[:E] = inputs[keep].ravel()
    packed = packed.reshape(NCORES, per_core)

    nb = Qb * 128 * TB
    arrs = {"xb": np.ascontiguousarray(packed[:, :nb]).reshape(NCORES * Qb, 128, TB)}
    if Ts:
        arrs["xt"] = np.ascontiguousarray(packed[:, nb:]).reshape(NCORES * 128, Ts)

    out = _run_packed(Qb, TB, Ts, arrs)  # [8*128, Q]
    total = out.sum(dtype=np.float64) - 0.5 * n_pad
    return np.asarray(np.float32(total / denom))
